# revision 1
# baseline (speedup 1.0000x reference)
"""Trainium2 Bass kernel for nn_BiLSTM_CRF (CRF negative log-likelihood loss).

Problem: loss = mean_b( logZ_b - gold_b ) for a linear-chain CRF with
B=512 sequences, T=512 steps, K=128 tags (START=126, STOP=127).

Algorithm (per core, data-parallel over batch, 64 sequences/core):
  The log-semiring forward scan is computed in the exp domain so each step
  is one 128x128x64 TensorE matmul with a *fixed* stationary weight
  W = exp(transitions^T - c), where c is a constant per-step shift that
  keeps exp-domain magnitudes in fp32/bf16 range (the per-step log-growth
  of the partition function is ~c; measured drift stays within +-7 log
  units over all 512 steps, far inside bf16/fp32 exponent range):

      A_0 = onehot(START);  A_{t+1} = exp(feats_t) ⊙ (W @ A_t)
      logZ = log(colsum(A_T ⊙ exp(T[STOP,:] - c))) + (T+1)*c

  Gold-path score splits into:
    - emit  = sum_t feats[b,t,tags[b,t]]      -> on device (touches feats):
      one fused DVE scalar_tensor_tensor per 128-row block:
      (iota_k == tag_p) * feats_nat with accum_out giving the free-dim sum.
      Emit ops are interleaved 1:2 with scan steps so they fill the DVE
      gaps between the scan's PSUM-evacuation multiplies.
    - trans = sum_t T[tag_t,tag_{t-1}] (+STOP) -> on host (64KB table gather).

feats is shipped twice in bf16 (transposed [K, t-major(T,B)] for the scan's
matmul/exp pipeline, natural [B*T, K] for emit) -- 16MB/core of DMA, fully
hidden under the ~512-step scan chain.

The final mean over batch is a host-side sum of the 8 per-core partials.
"""

import numpy as np
import ml_dtypes

import concourse.bass as bass
from concourse import bacc
import concourse.mybir as mybir
import concourse.tile as tile
from concourse.tile import add_dep_helper
from concourse.alu_op_type import AluOpType

B, T, K = 512, 512, 128
NCORES = 8
BPC = B // NCORES  # 64 sequences per core
START, STOP = K - 2, K - 1

# Constant per-step shift: E[logZ]/T measured on the problem's data
# distribution (randn feats/transitions). Any value within ~0.1 of the true
# mean growth keeps the scan in range; measured drift with this value is
# [-6.7, +5.9] log units.
C_SHIFT = 5.826096

TSEG = 32               # scan timesteps per exp() segment
NSEG = T // TSEG
NBLK = BPC * T // 128   # 256 natural-layout 128-row blocks for emit score
BLK_GRP = 8             # natural blocks DMA'd together
F32 = mybir.dt.float32
BF16 = mybir.dt.bfloat16

_NC_CACHE = {}


def build_kernel():
    key = "nc"
    if key in _NC_CACHE:
        return _NC_CACHE[key]
    nc = bacc.Bacc(None, target_bir_lowering=False)
    AF = mybir.ActivationFunctionType

    featsT_d = nc.dram_tensor("featsT", [K, T * BPC], BF16, kind="ExternalInput")
    featsN_d = nc.dram_tensor("featsN", [BPC * T, K], BF16, kind="ExternalInput")
    tags_d = nc.dram_tensor("tagsT", [128, NBLK], BF16, kind="ExternalInput")
    trans_d = nc.dram_tensor("transT", [K, K], F32, kind="ExternalInput")
    out_d = nc.dram_tensor("out", [1, BPC], F32, kind="ExternalOutput")
    emit_d = nc.dram_tensor("emitcols", [128, NBLK], F32, kind="ExternalOutput")

    with tile.TileContext(nc) as tc:
        with (
            tc.tile_pool(name="const", bufs=1) as cpool,
            tc.tile_pool(name="big", bufs=1) as bigpool,
            tc.tile_pool(name="seg", bufs=2) as segpool,
            tc.tile_pool(name="nat", bufs=4) as natpool,
            tc.tile_pool(name="apool", bufs=3) as apool,
            tc.tile_pool(name="scr", bufs=8) as scrpool,
            tc.tile_pool(name="psum", bufs=3, space="PSUM") as psum_pool,
            tc.tile_pool(name="psumf", bufs=1, space="PSUM") as psum_fin,
        ):
            # ---- constants ----
            # transT input is transitions^T - c (host pre-shifted), so W and
            # stopcol are both exp() of it; logZ = logS + (T+1)*c on host.
            transT_s = cpool.tile([K, K], F32)
            nc.sync.dma_start(out=transT_s, in_=trans_d[:])
            W = cpool.tile([K, K], BF16)  # [prev, next] = exp(T^T - c)
            nc.scalar.activation(W, transT_s, AF.Exp)
            stopcol = cpool.tile([K, 1], F32)  # exp(T[STOP, k] - c) per partition k
            nc.scalar.activation(stopcol, transT_s[:, STOP : STOP + 1], AF.Exp)
            ones_b = cpool.tile([K, 1], BF16)
            nc.vector.memset(ones_b, 1.0)
            iota_k = cpool.tile([K, K], BF16)  # iota_k[p, j] = j
            nc.gpsimd.iota(
                iota_k,
                pattern=[[1, K]],
                base=0,
                channel_multiplier=0,
                allow_small_or_imprecise_dtypes=True,
            )
            emit_cols = bigpool.tile([128, NBLK], F32)

            # ---- resident transposed feats, t-major: col = t*BPC + b ----
            # Chunked plain DMAs so segment 0 is ready within a few us;
            # segment 0 itself lands in 4 sub-chunks so the scan can start
            # as soon as the first 8 timesteps are in.
            featsT = bigpool.tile([K, T * BPC], BF16)
            seg_cols = TSEG * BPC
            for q in range(4):
                sub = seg_cols // 4
                nc.sync.dma_start(
                    out=featsT[:, q * sub : (q + 1) * sub],
                    in_=featsT_d[:, q * sub : (q + 1) * sub],
                )
            tags_s = cpool.tile([128, NBLK], BF16)
            nc.sync.dma_start(out=tags_s, in_=tags_d[:])
            for s in range(1, NSEG):
                nc.sync.dma_start(
                    out=featsT[:, s * seg_cols : (s + 1) * seg_cols],
                    in_=featsT_d[:, s * seg_cols : (s + 1) * seg_cols],
                )

            # natural-layout feats blocks for the emit score (scalar engine
            # HWDGE queue so the sync queue stays on the scan-critical loads)
            nat_tiles = []
            for g in range(NBLK // BLK_GRP):
                nat = natpool.tile([128, BLK_GRP, K], BF16)
                nc.scalar.dma_start(
                    out=nat,
                    in_=featsN_d[
                        g * BLK_GRP * 128 : (g + 1) * BLK_GRP * 128, :
                    ].rearrange("(j p) k -> p j k", j=BLK_GRP),
                )
                nat_tiles.append(nat)

            # ---- A0 = onehot(START): fill 1.0 where partition == START ----
            # Two half-batch chains (32 seqs each) interleave so one chain's
            # DVE multiply overlaps the other's matmul latency.
            HB = BPC // 2
            A_half = []
            for h in range(2):
                Ah = apool.tile([K, HB], BF16, name=f"A0_{h}", tag=f"a0_{h}")
                nc.gpsimd.memset(Ah, 0.0)
                nc.gpsimd.affine_select(
                    out=Ah,
                    in_=Ah,
                    compare_op=AluOpType.not_equal,
                    fill=1.0,
                    base=-START,
                    channel_multiplier=1,
                    pattern=[[0, HB]],
                )
                A_half.append(Ah)

            # ---- the scan, with emit ops interleaved 1 per 2 steps ----
            # An explicit (non-sem) scheduler dep from each emit op onto the
            # preceding scan multiply keeps the DVE queue alternating
            # scan/emit; without it the scheduler front-loads all 256 emit
            # ops, stalling the scan chain ~90us.
            def emit_op(col, after_inst):
                g, j = divmod(col, BLK_GRP)
                scr = scrpool.tile([128, K], BF16, name="scr")
                ei = nc.vector.scalar_tensor_tensor(
                    out=scr,
                    in0=iota_k,
                    scalar=tags_s[:, col : col + 1],
                    in1=nat_tiles[g][:, j, :],
                    op0=AluOpType.is_equal,
                    op1=AluOpType.mult,
                    accum_out=emit_cols[:, col : col + 1],
                )
                if after_inst is not None:
                    add_dep_helper(
                        ei.ins, after_inst.ins, sync=False,
                        reason="spread emit over scan gaps",
                    )

            emit_idx = 0
            for s in range(NSEG):
                expF = segpool.tile([K, TSEG * BPC], F32)
                if s == 0:
                    for q in range(4):
                        sub = seg_cols // 4
                        nc.scalar.activation(
                            expF[:, q * sub : (q + 1) * sub],
                            featsT[:, q * sub : (q + 1) * sub],
                            AF.Exp,
                        )
                else:
                    nc.scalar.activation(
                        expF, featsT[:, s * seg_cols : (s + 1) * seg_cols], AF.Exp
                    )
                for ti in range(TSEG):
                    mi = None
                    for h in range(2):
                        psum_M = psum_pool.tile([K, HB], F32, name=f"pm{h}")
                        nc.tensor.matmul(
                            psum_M, W, A_half[h], start=True, stop=True
                        )
                        A_new = apool.tile(
                            [K, HB], BF16, name=f"A_new{h}", tag=f"a{h}"
                        )
                        mi = nc.vector.tensor_mul(
                            A_new,
                            psum_M,
                            expF[:, ti * BPC + h * HB : ti * BPC + (h + 1) * HB],
                        )
                        A_half[h] = A_new
                    t_global = s * TSEG + ti
                    if t_global % 2 == 1 and emit_idx < NBLK:
                        emit_op(emit_idx, mi)
                        emit_idx += 1
            while emit_idx < NBLK:
                emit_op(emit_idx, None)
                emit_idx += 1

            # ---- finalize: logS = log(colsum(A ⊙ stopcol)) ----
            Afin = apool.tile([K, BPC], BF16)
            for h in range(2):
                nc.vector.tensor_scalar_mul(
                    Afin[:, h * HB : (h + 1) * HB], A_half[h], stopcol
                )
            psum_S = psum_fin.tile([1, BPC], F32)
            nc.tensor.matmul(psum_S, ones_b, Afin, start=True, stop=True)
            logS = cpool.tile([1, BPC], F32)
            nc.scalar.activation(logS, psum_S, AF.Ln)
            nc.sync.dma_start(out=out_d[:], in_=logS)
            nc.sync.dma_start(out=emit_d[:], in_=emit_cols)

    nc.compile()
    nc.finalize()
    _NC_CACHE[key] = nc
    return nc


def prep_inputs(feats, tags, transitions):
    """Host-side marshalling: slice per core, cast bf16, build both layouts."""
    feats_bf = np.asarray(feats, dtype=np.float32).astype(ml_dtypes.bfloat16)
    tags64 = np.asarray(tags).astype(np.int64)
    transT = np.ascontiguousarray(
        np.asarray(transitions, dtype=np.float32).T - np.float32(C_SHIFT)
    )
    in_maps = []
    for c in range(NCORES):
        fc = feats_bf[c * BPC : (c + 1) * BPC]  # [BPC, T, K]
        fT = np.ascontiguousarray(fc.transpose(2, 1, 0).reshape(K, T * BPC))
        fN = np.ascontiguousarray(fc.reshape(BPC * T, K))
        tg = np.ascontiguousarray(
            tags64[c * BPC : (c + 1) * BPC]
            .reshape(NBLK, 128)
            .T.astype(ml_dtypes.bfloat16)
        )
        in_maps.append({"featsT": fT, "featsN": fN, "tagsT": tg, "transT": transT})
    return in_maps, tags64


def combine_outputs(results, tags64, transitions):
    """Host-side: per-core logS/emit partials + trans gold score -> loss."""
    Trf = np.asarray(transitions, dtype=np.float64)
    ext = np.concatenate([np.full((B, 1), START, np.int64), tags64], axis=1)
    trans_gold = Trf[ext[:, 1:], ext[:, :-1]].sum(axis=1) + Trf[STOP, ext[:, -1]]
    total = 0.0
    for c in range(NCORES):
        logS = results[c]["out"][0].astype(np.float64)  # [BPC]
        ecols = results[c]["emitcols"].astype(np.float64)  # [128, NBLK]
        emit_b = ecols.sum(axis=0).reshape(BPC, 4).sum(axis=1)
        logZ = logS + (T + 1) * C_SHIFT
        total += float(np.sum(logZ - emit_b - trans_gold[c * BPC : (c + 1) * BPC]))
    return np.asarray(total / B, dtype=np.float32)


def kernel(feats, tags, transitions):
    from concourse.bass_utils import run_bass_kernel_spmd

    nc = build_kernel()
    in_maps, tags64 = prep_inputs(feats, tags, transitions)
    res = run_bass_kernel_spmd(nc, in_maps, list(range(NCORES)))
    return combine_outputs(res.results, tags64, transitions)


if __name__ == "__main__":
    nc = build_kernel()
    print("kernel built and compiled OK")



# revision 3
# speedup vs baseline: 1.7125x; 1.7125x over previous
"""Trainium2 Bass kernel for nn_BiLSTM_CRF (CRF negative log-likelihood loss).

Problem: loss = mean_b( logZ_b - gold_b ) for a linear-chain CRF with
B=512 sequences, T=512 steps, K=128 tags (START=126, STOP=127).

The partition function is a bilinear form through the chain:

    Z' = beta_t^T alpha_t   for any meeting point t, where
    alpha_{t+1} = D_t M alpha_t          (forward,  alpha_0 = e_START)
    beta_t      = M^T D_t beta_{t+1}     (backward, beta_T  = s)

with M[next,prev] = exp(transitions[next,prev] - c), D_t = diag(exp(feat_t)),
s = exp(transitions[STOP,:] - c).  The scan is latency-bound on TRN2 (each
step is a PSUM round trip: matmul -> DVE multiply -> matmul, ~0.5us fixed
latency), so running the forward scan over t=0..255 *concurrently* with the
backward scan over t=511..256 halves the sequential depth vs a pure forward
pass: 256 chained round trips instead of 512.  Both chains share the PE
(alternating stationaries Wf = exp(T^T - c), Wb = exp(T - c)) and the DVE
(one PSUM-evacuating multiply per chain per slot).

The constant per-step shift c keeps exp-domain magnitudes in range
(measured drift +-7 log units over 512 steps; each half drifts less).

Meeting: Z' = gamma_256^T (M alpha_256) with gamma_256 = E_256 * beta_257
(the backward chain's natural state), so the tail is one extra matmul and
one multiply; the [K,64] product ships to the host, which does the final
column-sum + log.  Gold-path score (emission gather + transition lookups,
O(B*T)) is computed on host in float64.

Per core (data-parallel over batch): 64 sequences, feats shipped once in
bf16, transposed [K, t-major(T,B)]; exp(feats) computed on ACT in 1024-col
segments streamed from both ends of the time axis.
"""

import numpy as np
import ml_dtypes

import concourse.bass as bass
from concourse import bacc
import concourse.mybir as mybir
import concourse.tile as tile

B, T, K = 512, 512, 128
NCORES = 8
BPC = B // NCORES  # 64 sequences per core
START, STOP = K - 2, K - 1
HALF = T // 2  # 256 timesteps per direction

# Constant per-step shift: E[logZ]/T measured on the problem's data
# distribution (randn feats/transitions).
C_SHIFT = 5.826096

SEGT = 16                # timesteps per exp() segment
SEGCOLS = SEGT * BPC     # 1024 columns per segment
NSEG = HALF // SEGT      # 16 segments per direction
F32 = mybir.dt.float32
BF16 = mybir.dt.bfloat16

_NC_CACHE = {}


def build_kernel():
    key = "nc"
    if key in _NC_CACHE:
        return _NC_CACHE[key]
    nc = bacc.Bacc(None, target_bir_lowering=False)
    AF = mybir.ActivationFunctionType

    featsT_d = nc.dram_tensor("featsT", [K, T * BPC], BF16, kind="ExternalInput")
    # [:, :K] = transitions^T - c (fwd stationary), [:, K:] = transitions - c
    trans2_d = nc.dram_tensor("trans2", [K, 2 * K], F32, kind="ExternalInput")
    fout_d = nc.dram_tensor("fout", [K, BPC], F32, kind="ExternalOutput")

    with tile.TileContext(nc) as tc:
        with (
            tc.tile_pool(name="const", bufs=1) as cpool,
            tc.tile_pool(name="big", bufs=1) as bigpool,
            tc.tile_pool(name="fseg", bufs=3) as fsegpool,
            tc.tile_pool(name="bseg", bufs=3) as bsegpool,
            tc.tile_pool(name="fa", bufs=3) as fapool,
            tc.tile_pool(name="ba", bufs=3) as bapool,
            tc.tile_pool(name="fps", bufs=2, space="PSUM") as fpsum,
            tc.tile_pool(name="bps", bufs=2, space="PSUM") as bpsum,
        ):
            # ---- constants (scalar-engine DMA queue, parallel with feats) ----
            trans2_s = cpool.tile([K, 2 * K], F32)
            nc.scalar.dma_start(out=trans2_s, in_=trans2_d[:])
            Wboth = cpool.tile([K, 2 * K], BF16)
            nc.scalar.activation(Wboth, trans2_s, AF.Exp)
            Wf = Wboth[:, :K]   # [prev, next] = exp(T[next,prev] - c)
            Wb = Wboth[:, K:]   # [next, prev] = exp(T[next,prev] - c)
            # mstart[k] = exp(T[k, START] - c) = M[:, START] (col of natural)
            mstart = cpool.tile([K, 1], F32)
            nc.scalar.activation(mstart, trans2_s[:, K + START : K + START + 1], AF.Exp)
            # stopcol[k] = exp(T[STOP, k] - c) = s (col STOP of transposed)
            stopcol = cpool.tile([K, 1], F32)
            nc.scalar.activation(stopcol, trans2_s[:, STOP : STOP + 1], AF.Exp)

            # ---- resident transposed feats, t-major: col = t*BPC + b ----
            # Chunks alternate low-end (fwd) / high-end (bwd) so both chains
            # start within ~1.5us of DMA-queue start.
            featsT = bigpool.tile([K, T * BPC], BF16)
            NC_TOT = T * BPC
            for s in range(NSEG):
                lo = s * SEGCOLS
                nc.sync.dma_start(
                    out=featsT[:, lo : lo + SEGCOLS],
                    in_=featsT_d[:, lo : lo + SEGCOLS],
                )
                hi = NC_TOT - (s + 1) * SEGCOLS
                nc.sync.dma_start(
                    out=featsT[:, hi : hi + SEGCOLS],
                    in_=featsT_d[:, hi : hi + SEGCOLS],
                )

            # ---- exp segments on ACT, alternating fwd/bwd ----
            # fseg[s] covers t in [16s, 16s+16); bseg[s] covers
            # t in [512-16(s+1), 512-16s)  (columns ascend in t).
            fsegs, bsegs = [], []
            for s in range(NSEG):
                fs = fsegpool.tile([K, SEGCOLS], F32, name=f"fseg{s % 3}")
                lo = s * SEGCOLS
                nc.scalar.activation(fs, featsT[:, lo : lo + SEGCOLS], AF.Exp)
                fsegs.append(fs)
                bs = bsegpool.tile([K, SEGCOLS], F32, name=f"bseg{s % 3}")
                hi = NC_TOT - (s + 1) * SEGCOLS
                nc.scalar.activation(bs, featsT[:, hi : hi + SEGCOLS], AF.Exp)
                bsegs.append(bs)

            def fcols(i):  # expF slice for fwd timestep t=i
                s, r = divmod(i, SEGT)
                return fsegs[s][:, r * BPC : (r + 1) * BPC]

            def bcols(i):  # expF slice for bwd timestep t=511-i
                t = T - 1 - i
                s = i // SEGT
                r = t - (T - SEGT * (s + 1))
                return bsegs[s][:, r * BPC : (r + 1) * BPC]

            # ---- chain init ----
            # alpha_1 = E_0 * M[:,START];  gamma_511 = E_511 * s
            A = fapool.tile([K, BPC], BF16, name="A")
            nc.vector.tensor_scalar_mul(A, fcols(0), mstart)
            G = bapool.tile([K, BPC], BF16, name="G")
            nc.vector.tensor_scalar_mul(G, bcols(0), stopcol)

            # ---- 255 paired slots: two independent latency chains ----
            for i in range(1, HALF):
                psF = fpsum.tile([K, BPC], F32, name="psF")
                nc.tensor.matmul(psF, Wf, A, start=True, stop=True)
                psB = bpsum.tile([K, BPC], F32, name="psB")
                nc.tensor.matmul(psB, Wb, G, start=True, stop=True)
                A = fapool.tile([K, BPC], BF16, name="A")
                nc.vector.tensor_mul(A, psF, fcols(i))
                G = bapool.tile([K, BPC], BF16, name="G")
                nc.vector.tensor_mul(G, psB, bcols(i))

            # ---- meet: Z' = gamma_256^T (M alpha_256), column-wise ----
            psF = fpsum.tile([K, BPC], F32, name="psFf")
            nc.tensor.matmul(psF, Wf, A, start=True, stop=True)
            Fout = cpool.tile([K, BPC], F32)
            nc.vector.tensor_mul(Fout, psF, G)
            nc.sync.dma_start(out=fout_d[:], in_=Fout)

    nc.compile()
    nc.finalize()
    _NC_CACHE[key] = nc
    return nc


def prep_inputs(feats, tags, transitions):
    """Host-side marshalling: slice per core, cast bf16, transpose t-major."""
    feats_bf = np.asarray(feats, dtype=np.float32).astype(ml_dtypes.bfloat16)
    tags64 = np.asarray(tags).astype(np.int64)
    Tr = np.asarray(transitions, dtype=np.float32)
    trans2 = np.ascontiguousarray(
        np.concatenate(
            [Tr.T - np.float32(C_SHIFT), Tr - np.float32(C_SHIFT)], axis=1
        )
    )
    in_maps = []
    for c in range(NCORES):
        fc = feats_bf[c * BPC : (c + 1) * BPC]  # [BPC, T, K]
        fT = np.ascontiguousarray(fc.transpose(2, 1, 0).reshape(K, T * BPC))
        in_maps.append({"featsT": fT, "trans2": trans2})
    return in_maps, tags64


def combine_outputs(results, tags64, feats, transitions):
    """Host: per-core bilinear products -> logZ; gold score in float64."""
    Trf = np.asarray(transitions, dtype=np.float64)
    ext = np.concatenate([np.full((B, 1), START, np.int64), tags64], axis=1)
    trans_gold = Trf[ext[:, 1:], ext[:, :-1]].sum(axis=1) + Trf[STOP, ext[:, -1]]
    featsf = np.asarray(feats, dtype=np.float64)
    emit_gold = (
        np.take_along_axis(featsf, tags64[:, :, None], axis=2)[..., 0].sum(axis=1)
    )
    total = 0.0
    for c in range(NCORES):
        F = results[c]["fout"].astype(np.float64)  # [K, BPC]
        logZ = np.log(F.sum(axis=0)) + (T + 1) * C_SHIFT
        sl = slice(c * BPC, (c + 1) * BPC)
        total += float(np.sum(logZ - trans_gold[sl] - emit_gold[sl]))
    return np.asarray(total / B, dtype=np.float32)


def kernel(feats, tags, transitions):
    from concourse.bass_utils import run_bass_kernel_spmd

    nc = build_kernel()
    in_maps, tags64 = prep_inputs(feats, tags, transitions)
    res = run_bass_kernel_spmd(nc, in_maps, list(range(NCORES)))
    return combine_outputs(res.results, tags64, feats, transitions)


if __name__ == "__main__":
    nc = build_kernel()
    print("kernel built and compiled OK")


# revision 12
# speedup vs baseline: 1.7542x; 1.0244x over previous
"""Trainium2 Bass kernel for nn_BiLSTM_CRF (CRF negative log-likelihood loss).

Problem: loss = mean_b( logZ_b - gold_b ) for a linear-chain CRF with
B=512 sequences, T=512 steps, K=128 tags (START=126, STOP=127).

The partition function is a bilinear form through the chain:

    Z' = beta_t^T alpha_t   for any meeting point t, where
    alpha_{t+1} = D_t M alpha_t          (forward,  alpha_0 = e_START)
    beta_t      = M^T D_t beta_{t+1}     (backward, beta_T  = s)

with M[next,prev] = exp(transitions[next,prev] - c), D_t = diag(exp(feat_t)),
s = exp(transitions[STOP,:] - c).  The scan is latency-bound on TRN2 (each
step is a PSUM round trip: matmul -> DVE multiply -> matmul, ~0.53us fixed
latency, which also exactly matches the DVE queue occupancy of the two
evacuations), so running the forward scan over t=0..255 *concurrently* with
the backward scan over t=511..256 halves the sequential depth vs a pure
forward pass: 256 chained round trips instead of 512.  Both chains share
the PE (alternating stationaries Wf = exp(T^T - c), Wb = exp(T - c),
LdWeights overlaps the previous matmul) and the DVE (one PSUM-evacuating
multiply per chain per slot).

The constant per-step shift c keeps exp-domain magnitudes in range
(measured drift +-7 log units over 512 steps; each half drifts less).

Meeting: Z' = gamma_256^T (M alpha_256) with gamma_256 = E_256 * beta_257
(the backward chain's natural state): gamma_256 (bf16) and the final
matmul's PSUM (f32) ship straight to DRAM; the host does the dot + log.
Gold-path score (emission gather + transition lookups, O(B*T)) is computed
on host in float64.  W ships pre-exponentiated so the first feats exp is
never queued behind transition DMA on the ACT engine.

Per core (data-parallel over batch): 64 sequences, feats shipped once in
bf16, transposed [K, t-major(T,B)]; exp(feats) computed on ACT in segments
streamed from both ends of the time axis (512-col first segments so the
chains start ~1us after the first DMA lands).
"""

import numpy as np
import ml_dtypes

import concourse.bass as bass
from concourse import bacc
import concourse.mybir as mybir
import concourse.tile as tile

B, T, K = 512, 512, 128
NCORES = 8
BPC = B // NCORES  # 64 sequences per core
START, STOP = K - 2, K - 1
HALF = T // 2  # 256 timesteps per direction

# Constant per-step shift: E[logZ]/T measured on the problem's data
# distribution (randn feats/transitions).
C_SHIFT = 5.826096

# Per-direction exp/DMA segment sizes in timesteps (sum = 256): small lead
# segments let the chains start early; big ones amortize boundary costs.
SEG_STEPS = [8, 24] + [32] * 7
F32 = mybir.dt.float32
BF16 = mybir.dt.bfloat16

_NC_CACHE = {}


def build_kernel():
    key = "nc"
    if key in _NC_CACHE:
        return _NC_CACHE[key]
    nc = bacc.Bacc(None, target_bir_lowering=False)
    AF = mybir.ActivationFunctionType

    featsT_d = nc.dram_tensor("featsT", [K, T * BPC], BF16, kind="ExternalInput")
    # [:, :K] = exp(transitions^T - c) (fwd stationary), [:, K:] = exp(T - c)
    wexp_d = nc.dram_tensor("wexp", [K, 2 * K], BF16, kind="ExternalInput")
    fout_d = nc.dram_tensor("fout", [K, BPC], F32, kind="ExternalOutput")

    seg_cols = [s * BPC for s in SEG_STEPS]
    seg_lo = np.cumsum([0] + seg_cols).tolist()  # fwd segment column offsets

    with tile.TileContext(nc) as tc:
        with (
            tc.tile_pool(name="const", bufs=1) as cpool,
            tc.tile_pool(name="big", bufs=1) as bigpool,
            tc.tile_pool(name="fseg", bufs=3) as fsegpool,
            tc.tile_pool(name="bseg", bufs=3) as bsegpool,
            tc.tile_pool(name="fa", bufs=3) as fapool,
            tc.tile_pool(name="ba", bufs=3) as bapool,
            tc.tile_pool(name="fps", bufs=2, space="PSUM") as fpsum,
            tc.tile_pool(name="bps", bufs=2, space="PSUM") as bpsum,
        ):
            # ---- constants (scalar-engine DMA queue, parallel with feats) ----
            Wboth = cpool.tile([K, 2 * K], BF16)
            nc.scalar.dma_start(out=Wboth, in_=wexp_d[:])
            Wf = Wboth[:, :K]
            Wb = Wboth[:, K:]

            # ---- resident transposed feats, t-major: col = t*BPC + b ----
            # One DMA per segment, alternating low-end (fwd) / high-end (bwd).
            featsT = bigpool.tile([K, T * BPC], BF16)
            NC_TOT = T * BPC
            for s in range(len(seg_cols)):
                lo = seg_lo[s]
                nc.sync.dma_start(
                    out=featsT[:, lo : lo + seg_cols[s]],
                    in_=featsT_d[:, lo : lo + seg_cols[s]],
                )
                hi = NC_TOT - lo - seg_cols[s]
                nc.sync.dma_start(
                    out=featsT[:, hi : hi + seg_cols[s]],
                    in_=featsT_d[:, hi : hi + seg_cols[s]],
                )

            # ---- exp segments on ACT, alternating fwd/bwd ----
            # fseg[s] covers fwd timesteps [seg_lo[s], seg_lo[s]+SEG_STEPS[s]);
            # bseg[s] covers the mirrored range at the top (columns ascend in t).
            SEGMAX = max(seg_cols)
            fsegs, bsegs = [], []
            for s in range(len(seg_cols)):
                fs = fsegpool.tile([K, SEGMAX], F32, name="fs")[:, : seg_cols[s]]
                lo = seg_lo[s]
                nc.scalar.activation(fs, featsT[:, lo : lo + seg_cols[s]], AF.Exp)
                fsegs.append(fs)
                bs = bsegpool.tile([K, SEGMAX], F32, name="bs")[:, : seg_cols[s]]
                hi = NC_TOT - lo - seg_cols[s]
                nc.scalar.activation(bs, featsT[:, hi : hi + seg_cols[s]], AF.Exp)
                bsegs.append(bs)

            step_seg = []  # fwd step i -> (segment index, column offset)
            for s, n in enumerate(SEG_STEPS):
                for r in range(n):
                    step_seg.append((s, r * BPC))

            def fcols(i):  # expF slice for fwd timestep t=i
                s, off = step_seg[i]
                return fsegs[s][:, off : off + BPC]

            def bcols(i):  # expF slice for bwd timestep t=511-i
                s, off = step_seg[i]
                w = seg_cols[s]
                return bsegs[s][:, w - off - BPC : w - off]

            # ---- chain init ----
            # log(M[:,START]) / log(s) are host-folded into the t=0 / t=511
            # feats columns, so alpha_1 / gamma_511 come out of the exp
            # segments directly; the copies just cast f32 -> bf16.
            A = fapool.tile([K, BPC], BF16, name="A")
            nc.vector.tensor_copy(A, fcols(0))
            G = bapool.tile([K, BPC], BF16, name="G")
            nc.vector.tensor_copy(G, bcols(0))

            # ---- 255 paired slots: two independent latency chains ----
            for i in range(1, HALF):
                psF = fpsum.tile([K, BPC], F32, name="psF")
                nc.tensor.matmul(psF, Wf, A, start=True, stop=True)
                psB = bpsum.tile([K, BPC], F32, name="psB")
                nc.tensor.matmul(psB, Wb, G, start=True, stop=True)
                A = fapool.tile([K, BPC], BF16, name="A")
                nc.vector.tensor_mul(A, psF, fcols(i))
                G = bapool.tile([K, BPC], BF16, name="G")
                nc.vector.tensor_mul(G, psB, bcols(i))

            # ---- meet: Z' = gamma_256^T (M alpha_256), column sum + log on host ----
            psF = fpsum.tile([K, BPC], F32, name="psFf")
            nc.tensor.matmul(psF, Wf, A, start=True, stop=True)
            Fout = cpool.tile([K, BPC], F32)
            nc.vector.tensor_mul(Fout, psF, G)
            nc.sync.dma_start(out=fout_d[:], in_=Fout)

    nc.compile()
    nc.finalize()
    _NC_CACHE[key] = nc
    return nc


def prep_inputs(feats, tags, transitions):
    """Host-side marshalling: slice per core, cast bf16, transpose t-major.

    The chain-endpoint transition vectors (log M[:,START], log s, each with
    the -c shift) are folded into the t=0 / t=511 feats columns so the device
    init is a plain copy out of the exp segment.
    """
    featsf = np.asarray(feats, dtype=np.float32).copy()
    tags64 = np.asarray(tags).astype(np.int64)
    Tr = np.asarray(transitions, dtype=np.float32)
    c32 = np.float32(C_SHIFT)
    featsf[:, 0, :] += Tr[:, START] - c32
    featsf[:, T - 1, :] += Tr[STOP, :] - c32
    feats_bf = featsf.astype(ml_dtypes.bfloat16)
    wexp = np.ascontiguousarray(
        np.concatenate(
            [np.exp(Tr.T - c32), np.exp(Tr - c32)], axis=1
        ).astype(ml_dtypes.bfloat16)
    )
    in_maps = []
    for c in range(NCORES):
        fc = feats_bf[c * BPC : (c + 1) * BPC]  # [BPC, T, K]
        fT = np.ascontiguousarray(fc.transpose(2, 1, 0).reshape(K, T * BPC))
        in_maps.append({"featsT": fT, "wexp": wexp})
    return in_maps, tags64


def combine_outputs(results, tags64, feats, transitions):
    """Host: per-core bilinear products -> logZ; gold score in float64."""
    Trf = np.asarray(transitions, dtype=np.float64)
    ext = np.concatenate([np.full((B, 1), START, np.int64), tags64], axis=1)
    trans_gold = Trf[ext[:, 1:], ext[:, :-1]].sum(axis=1) + Trf[STOP, ext[:, -1]]
    featsf = np.asarray(feats, dtype=np.float64)
    emit_gold = (
        np.take_along_axis(featsf, tags64[:, :, None], axis=2)[..., 0].sum(axis=1)
    )
    total = 0.0
    for c in range(NCORES):
        F = results[c]["fout"].astype(np.float64)  # [K, BPC] gamma * (M alpha)
        logZ = np.log(F.sum(axis=0)) + (T + 1) * C_SHIFT
        sl = slice(c * BPC, (c + 1) * BPC)
        total += float(np.sum(logZ - trans_gold[sl] - emit_gold[sl]))
    return np.asarray(total / B, dtype=np.float32)


def kernel(feats, tags, transitions):
    from concourse.bass_utils import run_bass_kernel_spmd

    nc = build_kernel()
    in_maps, tags64 = prep_inputs(feats, tags, transitions)
    res = run_bass_kernel_spmd(nc, in_maps, list(range(NCORES)))
    return combine_outputs(res.results, tags64, feats, transitions)


if __name__ == "__main__":
    nc = build_kernel()
    print("kernel built and compiled OK")
